# revision 41
# baseline (speedup 1.0000x reference)
"""Trainium2 Bass kernel for nn_CGCN (relational GCN with distance-weighted
message passing + mirror-descent relation coefficients), 8-core SPMD.

Self-contained: takes full inputs, shards internally, returns full outputs.

The SPMD dispatch is transfer-bound (axon tunnel ~70 MB/s), so the host->device
payload is minimized: x ships as per-node int8 + bf16 scales (dequantized on
device), gather indices ship unreplicated (16-partition payload, replicated to
128 partitions by on-device DMAs), edge metadata ships as int8/bf16, and the
iota/identity constants are generated on device. The jitted dispatch closure is
built once and cached so warm calls skip retrace/relower.
"""
import sys
for _p in ("/opt/trn_rl_repo", "/root/.axon_site/_ro/trn_rl_repo"):
    if _p not in sys.path:
        sys.path.insert(0, _p)
import numpy as np
import ml_dtypes

from concourse import bacc, bass, mybir, tile
from concourse import library_config
from concourse.bass_utils import run_bass_kernel_spmd

bf16 = ml_dtypes.bfloat16
FP = mybir.dt.float32
BF = mybir.dt.bfloat16
I8 = mybir.dt.int8
I16 = mybir.dt.int16
I32 = mybir.dt.int32
Alu = mybir.AluOpType
Act = mybir.ActivationFunctionType
AX = mybir.AxisListType

N = 50000
NF = 500
NFP = 512
NH = 128
NC = 16
NR = 3
E = 300000
NPAD = 50176          # 392 tiles of 128
NCORES = 8
TPC = 49              # tiles per core
GPL = 7               # groups per layer (tile groups)
TPG = 7               # tiles per group
BPG = TPG * NR        # bins per group = 21
SLOT = 512            # slots per half-bin (lo/hi)
CHUNKS = 8            # chunks per bin (4 lo + 4 hi)
HALF = 25088          # row split for int16 indices
SPC = NPAD // NCORES  # nodes per core slice = 6272
ALPHA = 0.1
RG_GROUPS = 56        # rescale groups of 7 gtiles (392 total)
NQ = 4                # SWDGE queues used for gathers


def wrap16(ids):
    # ids [..., 512] -> gpsimd wrapped layout [..., 16, 32] (unreplicated)
    sh = ids.shape[:-1]
    w = ids.reshape(*sh, 32, 16)
    return np.ascontiguousarray(np.swapaxes(w, -1, -2)).astype(np.int16)


def prepare(x, edge_index, W1, b1, W2, b2):
    ei = np.asarray(edge_index)
    deg = np.stack([np.clip(np.bincount(ei[r, 0], minlength=N).astype(np.float32), 1.0, None) for r in range(NR)])
    d05 = deg**-0.5; d025 = deg**-0.25
    idx_all = np.zeros((NR, 392, 2, SLOT), np.int64)
    cid_all = np.zeros((NR, 392, 2, SLOT), np.int64)
    ecl_all = np.full((NR, 392, 2, SLOT), -1, np.int8)
    wq_all = np.zeros((NR, 392, 2, SLOT), bf16)
    cnt_all = np.zeros((NR, 392, 2), np.int64)
    for r in range(NR):
        row, col = ei[r, 0].astype(np.int64), ei[r, 1].astype(np.int64)
        tilev = col >> 7
        hi = (row >= HALF).astype(np.int64)
        key = tilev * 2 + hi
        order = np.argsort(key, kind="stable")
        ks = key[order]
        cnt = np.bincount(ks, minlength=784)
        cnt_all[r] = cnt.reshape(392, 2)
        off = np.concatenate([[0], np.cumsum(cnt)])[:-1]
        pos = np.arange(len(ks)) - np.repeat(off, cnt)
        assert pos.max() < SLOT, pos.max()
        rs, cs = row[order], col[order]
        q = (d05[r][rs] * d05[r][cs] / d025[r][rs]).astype(np.float32)
        t_s, h_s = ks >> 1, ks & 1
        idx_all[r, t_s, h_s, pos] = rs - h_s * HALF
        cid_all[r, t_s, h_s, pos] = cs - (t_s // TPC) * SPC  # col idx in owning core slice
        ecl_all[r, t_s, h_s, pos] = (cs & 127).astype(np.int8)
        wq_all[r, t_s, h_s, pos] = q.astype(bf16)
    # x: per-node int8 quantization, shipped transposed [feat, node]
    xf = np.asarray(x, np.float32)
    amax = np.maximum(np.abs(xf).max(axis=1), 1e-12)
    sc = (amax / 127.0).astype(bf16)                     # shipped scale (bf16)
    inv = (127.0 / amax).astype(np.float32)
    xq = np.rint(xf * inv[:, None]).astype(np.int8)      # [N, NF]
    xqT = np.zeros((NF, NPAD), np.int8); xqT[:, :N] = xq.T
    scp = np.zeros((1, NPAD), bf16); scp[0, :N] = sc
    # globally-concatenated per-core arrays (axis 0 = core), ready for dispatch
    g = dict(
        xq=np.empty((NCORES * NF, SPC), np.int8),
        xsc=np.empty((NCORES * 1, SPC), bf16),
        gidx=np.empty((NCORES * GPL, 16, NR, TPG, 128), np.int16),
        ecl=np.empty((NCORES * GPL, 128, NR, TPG, CHUNKS), np.int8),
        wq=np.empty((NCORES * GPL, 128, NR, TPG, CHUNKS), bf16),
        row0=np.arange(NCORES, dtype=np.int32).reshape(NCORES, 1) * SPC,
    )
    for c in range(NCORES):
        sl = slice(c * TPC, (c + 1) * TPC)
        idx_c = wrap16(idx_all[:, sl].reshape(NR * TPC * 2, SLOT)).reshape(NR, TPC, 2, 16, 32)
        cid_c = wrap16(cid_all[:, sl].reshape(NR * TPC * 2, SLOT)).reshape(NR, TPC, 2, 16, 32)

        def to_idx_layout(a):
            # [NR, TPC, 2, 16, 32] -> [GPL, 16, NR, TPG, 64]
            a = np.concatenate([a[:, :, 0], a[:, :, 1]], axis=-1)   # [NR, TPC, 16, 64]
            a = a.reshape(NR, GPL, TPG, 16, 64)
            return a.transpose(1, 3, 0, 2, 4)
        gc = slice(c * GPL, (c + 1) * GPL)
        g["gidx"][gc, :, :, :, 0:64] = to_idx_layout(idx_c)
        g["gidx"][gc, :, :, :, 64:128] = to_idx_layout(cid_c)
        # edge metadata in chunk layout [GPL, 128, NR, TPG, CHUNKS]
        def to_chunk_layout(a):
            a = a[:, sl].reshape(NR, GPL, TPG, CHUNKS, 128)
            return a.transpose(1, 4, 0, 2, 3)
        g["ecl"][gc] = to_chunk_layout(ecl_all)
        g["wq"][gc] = to_chunk_layout(wq_all)
        g["xq"][c * NF:(c + 1) * NF] = xqT[:, c * SPC:(c + 1) * SPC]
        g["xsc"][c] = scp[0, c * SPC:(c + 1) * SPC]
    return g


def build_program(n_groups=GPL):
    nc = bacc.Bacc("TRN2", target_bir_lowering=False, debug=False,
                   num_devices=NCORES, num_swdge_queues=NQ)

    # ---- external inputs ----
    xqT = nc.dram_tensor("xq", [NF, SPC], I8, kind="ExternalInput")
    xscT = nc.dram_tensor("xsc", [1, SPC], BF, kind="ExternalInput")
    W1s = nc.dram_tensor("W1s", [NFP // NCORES, NH], BF, kind="ExternalInput")
    wsmT = nc.dram_tensor("wsm", [130, NH], BF, kind="ExternalInput")
    d025sT = nc.dram_tensor("d025s", [NR, GPL, 128, 7], BF, kind="ExternalInput")
    cvecn = nc.dram_tensor("cvecn", [1, 64], FP, kind="ExternalInput")
    gidxT = nc.dram_tensor("gidx", [GPL, 16, NR, TPG, 128], I16, kind="ExternalInput")
    eclT = nc.dram_tensor("ecl", [GPL, 128, NR, TPG, CHUNKS], I8, kind="ExternalInput")
    wqT = nc.dram_tensor("wq", [GPL, 128, NR, TPG, CHUNKS], BF, kind="ExternalInput")
    row0T = nc.dram_tensor("row0", [1, 1], I32, kind="ExternalInput")

    out_all = nc.dram_tensor("out_all", [NPAD, 2 * NC], BF, kind="ExternalOutput")

    with tile.TileContext(nc) as tc:
        with (
            tc.tile_pool(name="per", bufs=1) as per,            # persistent
            tc.tile_pool(name="wk", bufs=3) as wk,              # rotating small
            tc.tile_pool(name="ps", bufs=3, space="PSUM") as psp,
            tc.tile_pool(name="pst", bufs=2, space="PSUM") as pstp,
            tc.tile_pool(name="psl", bufs=2, space="PSUM") as pslp,
            tc.tile_pool(name="dram", bufs=1, space="DRAM") as dr,
        ):
            nc.gpsimd.load_library(library_config.mlp)

            # ---- internal DRAM ----
            tabs = [dr.tile([NPAD, NH], BF, name=f"tab{r}") for r in range(NR)]
            mytabs = [dr.tile([SPC, NH], BF, name=f"mytab{r}") for r in range(NR)]
            h_slice = dr.tile([SPC, NH], BF, name="h_slice")
            h_fulls = [dr.tile([NPAD, NH], BF, name=f"h_full{i}", addr_space="Shared")
                       for i in range(2)]
            ar_in = dr.tile([1, 4], FP, name="ar_in")
            ar_outs = [dr.tile([1, 4], FP, name=f"ar_out{i}", addr_space="Shared")
                       for i in range(2)]
            w1i = dr.tile([NFP // NCORES, NH], BF, name="w1i")
            w1g = dr.tile([NFP, NH], BF, name="w1g", addr_space="Shared")
            d025i = dr.tile([NR, GPL, 128, 7], BF, name="d025i")
            d025g = dr.tile([NCORES, NR, GPL, 128, 7], BF, name="d025g",
                            addr_space="Shared")
            out_loc = dr.tile([SPC, 2 * NC], BF, name="out_loc")
            out_g = dr.tile([NPAD, 2 * NC], BF, name="out_g", addr_space="Shared")

            # ---- persistent SBUF ----
            it_f = per.tile([128, 128], I16)
            nc.gpsimd.iota(it_f[:], pattern=[[1, 128]], base=0, channel_multiplier=0)
            iota_b = per.tile([128, 128], BF)
            nc.vector.tensor_scalar(out=iota_b[:], in0=it_f[:], scalar1=0,
                                    scalar2=None, op0=Alu.add)
            it_d = per.tile([128, 128], I16)
            nc.gpsimd.iota(it_d[:], pattern=[[1, 128]], base=0, channel_multiplier=-1)
            ident = per.tile([128, 128], BF)
            nc.vector.tensor_scalar(out=ident[:], in0=it_d[:], scalar1=0,
                                    scalar2=None, op0=Alu.is_equal)
            ones1 = per.tile([1, 128], BF)
            nc.vector.memset(ones1[:], 1.0)
            eps_t = per.tile([128, 1], FP)
            nc.vector.memset(eps_t[:], 1e-4)
            cvec = per.tile([1, 64], FP)
            nc.sync.dma_start(cvec[:], cvecn[:, :])
            # replicated weights arrive as 1/8 slices; allgather on device
            nc.sync.dma_start(w1i[:, :], W1s[:, :])
            nc.gpsimd.collective_compute(
                "AllGather", Alu.bypass,
                replica_groups=[list(range(NCORES))],
                ins=[w1i[:].opt()], outs=[w1g[:].opt()],
            )
            nc.sync.dma_start(d025i[:], d025sT[:, :, :, :])
            nc.gpsimd.collective_compute(
                "AllGather", Alu.bypass,
                replica_groups=[list(range(NCORES))],
                ins=[d025i[:].opt()], outs=[d025g[:].opt()],
            )
            w1t = per.tile([128, 4, NH], BF)
            nc.sync.dma_start(w1t[:], w1g.rearrange("(k p) h -> p k h", p=128))
            w2t = per.tile([128, NC], BF)
            nc.sync.dma_start(w2t[:], wsmT[0:128, 0:NC])
            b1t = per.tile([1, NH], BF)
            nc.sync.dma_start(b1t[:], wsmT[128:129, :])
            b2t = per.tile([1, NC], BF)
            nc.sync.dma_start(b2t[:], wsmT[129:130, 0:NC])
            r0t = per.tile([1, 1], I32)
            nc.sync.dma_start(r0t[:], row0T[:, :])
            row0v = nc.values_load(r0t[0:1, 0:1].bitcast(I32).to_broadcast((1, 1)))

            xq8 = per.tile([128, 4, SPC], I8)
            nc.vector.memset(xq8[:, 3, :], 0)
            nc.sync.dma_start(xq8[:, 0:3, :],
                              xqT[0:384, :].rearrange("(k p) n -> p k n", p=128))
            nc.sync.dma_start(xq8[0:116, 3, :], xqT[384:NF, :])
            xscb = per.tile([1, SPC], BF)
            nc.sync.dma_start(xscb[:], xscT[:, :])
            scf = per.tile([128, SPC], BF)
            nc.gpsimd.partition_broadcast(scf[:], xscb[0:1, :])

            raw = per.tile([128, TPC, NH], BF)        # my slice post-relu
            spill = per.tile([128, GPL, TPG, NR, NH], BF)
            hrb = per.tile([128, NR, TPG, CHUNKS, NH], BF)
            nc.vector.memset(hrb[:], 0.0)             # trimmed gathers leave tails stale
            dist2g = per.tile([128, NR, TPG, CHUNKS], FP)
            ecl8 = per.tile([128, NR, TPG, CHUNKS], I8)
            ecl_f = per.tile([128, NR, TPG, CHUNKS], FP)
            mk_t = per.tile([128, NR, TPG, CHUNKS], FP)
            wq_b = per.tile([128, NR, TPG, CHUNKS], BF)
            idxg = per.tile([128, NR, TPG, 64], I16)
            cidxg = per.tile([128, NR, TPG, 64], I16)
            wbuf = per.tile([128, NR, TPG, CHUNKS], FP)
            s_acc = per.tile([128, 4], FP)
            s_row = per.tile([1, 4], FP)
            negT = per.tile([1, 64], FP)
            u_t = per.tile([1, 4], FP)
            uta = per.tile([1, 4], FP)
            fde = per.tile([1, 4], FP)
            ssum = per.tile([1, 1], FP)
            isr = per.tile([1, 1], FP)
            fi_t = per.tile([1, 1], FP)
            ub = per.tile([128, 4], FP)

            h_slice_r = h_slice.rearrange("(t p) h -> p t h", p=128)  # [128, TPC, NH]

            # ================= P0: layer 0 =================
            for t in range(TPC):
                xtile = wk.tile([128, 4, 128], BF, tag="xtile")
                for kc in range(4):
                    nc.vector.tensor_tensor(
                        out=xtile[:, kc, :], in0=xq8[:, kc, t * 128:(t + 1) * 128],
                        in1=scf[:, t * 128:(t + 1) * 128], op=Alu.mult)
                ps0 = psp.tile([128, NH], FP, tag="ps")
                for kc in range(4):
                    nc.tensor.matmul(ps0[:], lhsT=xtile[:, kc, :],
                                     rhs=w1t[:, kc, :], start=(kc == 0), stop=False)
                nc.tensor.matmul(ps0[:], lhsT=ones1[:], rhs=b1t[:],
                                 start=False, stop=True)
                nc.scalar.activation(raw[:, t, :], ps0[:], Act.Relu)
                nc.sync.dma_start(h_slice_r[:, t, :], raw[:, t, :])

            def allgather(i):
                nc.gpsimd.collective_compute(
                    "AllGather", Alu.bypass,
                    replica_groups=[list(range(NCORES))],
                    ins=[h_slice[:].opt()], outs=[h_fulls[i][:].opt()],
                )

            def rescale(i):
                h_full_r = h_fulls[i].rearrange("(t p) h -> p t h", p=128)
                for gp in range(RG_GROUPS):
                    hg = wk.tile([128, 7, NH], BF, tag="hg")
                    nc.sync.dma_start(hg[:], h_full_r[:, gp * 7:(gp + 1) * 7, :])
                    for r in range(NR):
                        dg = wk.tile([128, 7], BF, tag="dg")
                        nc.sync.dma_start(dg[:], d025g[gp // GPL, r, gp % GPL, :, :])
                        sg = wk.tile([128, 7, NH], BF, tag="sg")
                        nc.vector.tensor_tensor(
                            out=sg[:], in0=hg[:],
                            in1=dg[:].broadcast_to([128, 7, NH]),
                            op=Alu.mult)
                        tab_r = tabs[r].rearrange("(t p) h -> p t h", p=128)
                        nc.sync.dma_start(tab_r[:, gp * 7:(gp + 1) * 7, :], sg[:])
                for r in range(NR):
                    nc.sync.dma_start(mytabs[r][:, :],
                                      tabs[r][bass.ds(row0v, SPC), :])

            allgather(0)
            rescale(0)

            # pre-init rotating gather buffers (trimmed gathers leave tails stale)
            for _ in range(3):
                hcb0 = wk.tile([128, CHUNKS, NH], BF, tag="hcb")
                nc.vector.memset(hcb0[:], 0.0)

            # ================= layers =================
            qn = [0]
            for layer in (1, 2):
                nc.vector.memset(s_acc[:], 0.0)
                for g in range(n_groups):
                    # --- phase 1: gather + dist2 ---
                    for k in range(8):
                        nc.sync.dma_start(idxg[16 * k:16 * k + 16, :, :, :],
                                          gidxT[g, :, :, :, 0:64])
                        nc.sync.dma_start(cidxg[16 * k:16 * k + 16, :, :, :],
                                          gidxT[g, :, :, :, 64:128])
                    nc.sync.dma_start(ecl8[:], eclT[g, :, :, :, :])
                    nc.sync.dma_start(wq_b[:], wqT[g, :, :, :, :])
                    nc.vector.tensor_scalar(out=ecl_f[:], in0=ecl8[:], scalar1=0,
                                            scalar2=None, op0=Alu.add)
                    nc.vector.tensor_scalar(out=mk_t[:], in0=ecl_f[:], scalar1=0.0,
                                            scalar2=None, op0=Alu.is_ge)
                    for lt in range(TPG):
                        for r in range(NR):
                            for h, tab_h in ((0, tabs[r][0:HALF, :]),
                                             (1, tabs[r][HALF:NPAD, :])):
                                nc.gpsimd.dma_gather(
                                    out_ap=hrb[:, r, lt, 4 * h:4 * h + 4, :],
                                    in_ap=tab_h,
                                    idxs_ap=idxg[:, r, lt, 32 * h:32 * h + 32],
                                    num_idxs=SLOT, num_idxs_reg=SLOT,
                                    elem_size=NH,
                                    queue_num=qn[0] % NQ)
                                qn[0] += 1
                            hcb = wk.tile([128, CHUNKS, NH], BF, tag="hcb")
                            for h in (0, 1):
                                nc.gpsimd.dma_gather(
                                    out_ap=hcb[:, 4 * h:4 * h + 4, :],
                                    in_ap=mytabs[r][:, :],
                                    idxs_ap=cidxg[:, r, lt, 32 * h:32 * h + 32],
                                    num_idxs=SLOT, num_idxs_reg=SLOT,
                                    elem_size=NH,
                                    queue_num=qn[0] % NQ)
                                qn[0] += 1
                            diff = wk.tile([128, CHUNKS, NH], BF, tag="diff")
                            nc.vector.tensor_tensor(out=diff[:], in0=hrb[:, r, lt, :, :],
                                                    in1=hcb[:], op=Alu.subtract)
                            for c in range(CHUNKS):
                                sq = wk.tile([128, NH], BF, tag="sq")
                                nc.vector.scalar_tensor_tensor(
                                    out=sq[:], in0=diff[:, c, :], scalar=1.0,
                                    in1=diff[:, c, :], op0=Alu.mult, op1=Alu.mult,
                                    accum_out=dist2g[:, r, lt, c:c + 1])
                    # --- batch scalar pipeline ---
                    d_flat = dist2g[:].rearrange("p r t c -> p (r t c)")
                    Lt = wk.tile([128, NR * TPG * CHUNKS], FP, tag="Lt")
                    nc.scalar.activation(Lt[:], d_flat, Act.Ln, bias=eps_t[:])
                    rec = wk.tile([128, NR * TPG * CHUNKS], FP, tag="rec")
                    nc.scalar.activation(rec[:], Lt[:], Act.Exp, scale=-0.5)
                    sd = wk.tile([128, NR * TPG * CHUNKS], FP, tag="sd")
                    nc.scalar.activation(sd[:], Lt[:], Act.Exp, scale=0.5)
                    t2 = wk.tile([128, NR * TPG * CHUNKS], FP, tag="t2")
                    nc.scalar.activation(t2[:], rec[:], Act.Exp, scale=-2.0)
                    num = wk.tile([128, NR * TPG * CHUNKS], FP, tag="num")
                    nc.vector.tensor_scalar(out=num[:], in0=t2[:], scalar1=-1.0,
                                            scalar2=1.0, op0=Alu.mult, op1=Alu.add)
                    den = wk.tile([128, NR * TPG * CHUNKS], FP, tag="den")
                    nc.vector.tensor_scalar(out=den[:], in0=t2[:], scalar1=1.0,
                                            scalar2=None, op0=Alu.add)
                    idn = wk.tile([128, NR * TPG * CHUNKS], FP, tag="idn")
                    nc.vector.reciprocal(idn[:], den[:])
                    gg = wk.tile([128, NR * TPG * CHUNKS], FP, tag="gg")
                    nc.vector.tensor_tensor(out=gg[:], in0=num[:], in1=idn[:],
                                            op=Alu.mult)
                    w_flat = wbuf[:].rearrange("p r t c -> p (r t c)")
                    nc.vector.tensor_tensor(
                        out=w_flat, in0=gg[:],
                        in1=wq_b[:].rearrange("p r t c -> p (r t c)"),
                        op=Alu.mult)
                    sd_v = sd[:].rearrange("p (r t c) -> p r t c", r=NR, t=TPG)
                    for r in range(NR):
                        sms = wk.tile([128, TPG, CHUNKS], FP, tag="sms")
                        stm = wk.tile([128, 1], FP, tag="stm")
                        nc.vector.scalar_tensor_tensor(
                            out=sms[:], in0=sd_v[:, r, :, :], scalar=1.0,
                            in1=mk_t[:, r, :, :], op0=Alu.mult, op1=Alu.mult,
                            accum_out=stm[:])
                        nc.vector.tensor_tensor(out=s_acc[:, r:r + 1],
                                                in0=s_acc[:, r:r + 1],
                                                in1=stm[:], op=Alu.add)
                    # --- phase 2: scatter ---
                    for lt in range(TPG):
                        for r in range(NR):
                            pss = psp.tile([128, NH], FP, tag="ps")
                            for c in range(CHUNKS):
                                woh = wk.tile([128, 128], BF, tag="woh")
                                nc.vector.tensor_scalar(
                                    out=woh[:], in0=iota_b[:],
                                    scalar1=ecl_f[:, r, lt, c:c + 1],
                                    scalar2=wbuf[:, r, lt, c:c + 1],
                                    op0=Alu.is_equal, op1=Alu.mult)
                                nc.tensor.matmul(pss[:], lhsT=woh[:],
                                                 rhs=hrb[:, r, lt, c, :],
                                                 start=(c == 0), stop=(c == CHUNKS - 1))
                            nc.scalar.activation(spill[:, g, lt, r, :], pss[:], Act.Copy)

                # --- s_r reduce + allreduce ---
                sr_l = wk.tile([1, 4], FP, tag="srl")
                nc.gpsimd.tensor_reduce(out=sr_l[:], in_=s_acc[:],
                                        axis=AX.C, op=Alu.add)
                nc.sync.dma_start(ar_in[:, :], sr_l[:])
                nc.gpsimd.collective_compute(
                    "AllReduce", Alu.add,
                    replica_groups=[list(range(NCORES))],
                    ins=[ar_in[:].opt()], outs=[ar_outs[layer - 1][:].opt()],
                )
                nc.sync.dma_start(s_row[:], ar_outs[layer - 1][:, :])
                nc.vector.tensor_scalar(out=s_row[:], in0=s_row[:],
                                        scalar1=1.0 / E, scalar2=None, op0=Alu.mult)

                # --- mirror descent ---
                nc.vector.tensor_reduce(out=fi_t[:], in_=s_row[0:1, 0:3],
                                        axis=AX.X, op=Alu.add)
                nc.vector.tensor_scalar(out=fi_t[:], in0=fi_t[:], scalar1=2.0 / 9.0,
                                        scalar2=None, op0=Alu.add)
                nc.vector.reciprocal(isr[:], fi_t[:])
                nc.vector.tensor_scalar(out=negT[:], in0=cvec[:], scalar1=isr[0:1, 0:1],
                                        scalar2=None, op0=Alu.mult)
                nc.vector.memset(u_t[:], 1.0 / NR)
                for i in range(50):
                    nc.vector.scalar_tensor_tensor(
                        out=fde[0:1, 0:3], in0=u_t[0:1, 0:3], scalar=2.0 / 9.0,
                        in1=s_row[0:1, 0:3], op0=Alu.mult, op1=Alu.add)
                    nc.scalar.activation(uta[0:1, 0:3], fde[0:1, 0:3], Act.Exp,
                                         scale=negT[0:1, i:i + 1])
                    nc.vector.scalar_tensor_tensor(
                        out=uta[0:1, 0:3], in0=u_t[0:1, 0:3], scalar=1.0,
                        in1=uta[0:1, 0:3], op0=Alu.mult, op1=Alu.mult,
                        accum_out=ssum[:])
                    nc.vector.reciprocal(isr[:], ssum[:])
                    nc.vector.tensor_scalar(out=u_t[0:1, 0:3], in0=uta[0:1, 0:3],
                                            scalar1=isr[0:1, 0:1], scalar2=None,
                                            op0=Alu.mult)
                nc.vector.tensor_scalar(out=u_t[0:1, 0:3], in0=u_t[0:1, 0:3],
                                        scalar1=1.0 - ALPHA, scalar2=None,
                                        op0=Alu.mult)
                nc.gpsimd.partition_broadcast(ub[:, 0:4], u_t[0:1, 0:4])

                # --- combine ---
                for g in range(n_groups):
                    for lt in range(TPG):
                        t = g * TPG + lt
                        accf = wk.tile([128, NH], FP, tag="accf")
                        nc.vector.tensor_scalar(out=accf[:], in0=spill[:, g, lt, 0, :],
                                                scalar1=ub[:, 0:1], scalar2=None,
                                                op0=Alu.mult)
                        for r in (1, 2):
                            nc.vector.scalar_tensor_tensor(
                                out=accf[:], in0=spill[:, g, lt, r, :],
                                scalar=ub[:, r:r + 1], in1=accf[:],
                                op0=Alu.mult, op1=Alu.add)
                        hn = wk.tile([128, NH], BF, tag="hn")
                        nc.vector.scalar_tensor_tensor(
                            out=hn[:], in0=raw[:, t, :], scalar=ALPHA,
                            in1=accf[:], op0=Alu.mult, op1=Alu.add)
                        if layer == 1:
                            nc.sync.dma_start(h_slice_r[:, t, :], hn[:])
                        else:
                            pstt = pstp.tile([128, 128], BF, tag="pstT")
                            nc.tensor.transpose(pstt[:], hn[:], identity=ident[:])
                            h2T = wk.tile([128, 128], BF, tag="h2T")
                            nc.scalar.activation(h2T[:], pstt[:], Act.Copy)
                            psl = pslp.tile([128, NC], FP, tag="psl")
                            nc.tensor.matmul(psl[:], lhsT=h2T[:], rhs=w2t[:],
                                             start=True, stop=False)
                            nc.tensor.matmul(psl[:], lhsT=ones1[:], rhs=b2t[:],
                                             start=False, stop=True)
                            lgf = wk.tile([128, NC], FP, tag="lgf")
                            nc.scalar.activation(lgf[:], psl[:], Act.Copy)
                            lgb = wk.tile([128, NC], BF, tag="lgb")
                            nc.scalar.activation(lgb[:], psl[:], Act.Copy)
                            mx = wk.tile([128, 1], FP, tag="mx")
                            nc.vector.tensor_reduce(out=mx[:], in_=lgf[:],
                                                    axis=AX.X, op=Alu.max)
                            ngm = wk.tile([128, 1], FP, tag="ngm")
                            nc.vector.tensor_scalar(out=ngm[:], in0=mx[:],
                                                    scalar1=-1.0, scalar2=None,
                                                    op0=Alu.mult)
                            esc = wk.tile([128, NC], FP, tag="esc")
                            se = wk.tile([128, 1], FP, tag="se")
                            nc.scalar.activation(esc[:], lgf[:], Act.Exp,
                                                 bias=ngm[:], accum_out=se[:])
                            lse = wk.tile([128, 1], FP, tag="lse")
                            nc.scalar.activation(lse[:], se[:], Act.Ln)
                            mml = wk.tile([128, 1], FP, tag="mml")
                            nc.vector.tensor_tensor(out=mml[:], in0=mx[:],
                                                    in1=lse[:], op=Alu.add)
                            lsmb = wk.tile([128, NC], BF, tag="lsmb")
                            nc.vector.tensor_scalar(out=lsmb[:], in0=lgf[:],
                                                    scalar1=mml[:], scalar2=None,
                                                    op0=Alu.subtract)
                            nc.sync.dma_start(
                                out_loc[t * 128:(t + 1) * 128, NC:2 * NC], lgb[:])
                            nc.sync.dma_start(
                                out_loc[t * 128:(t + 1) * 128, 0:NC], lsmb[:])

                if layer == 1:
                    allgather(1)
                    rescale(1)

            # gather the full output onto every core; host fetches one replica
            nc.gpsimd.collective_compute(
                "AllGather", Alu.bypass,
                replica_groups=[list(range(NCORES))],
                ins=[out_loc[:].opt()], outs=[out_g[:].opt()],
            )
            nc.sync.dma_start(out_all[:, :], out_g[:, :])

    nc.compile()
    return nc


_CACHED = {}
LAST_SPMD_SECONDS = None


def _shared_inputs(W1, b1, W2, b2):
    W1T = np.zeros((NFP, NH), bf16); W1T[:NF, :] = np.asarray(W1).T.astype(bf16)
    wsm = np.zeros((130, NH), bf16)
    wsm[0:NH, 0:NC] = np.asarray(W2).T.astype(bf16)
    wsm[128, :] = np.asarray(b1).astype(bf16)
    wsm[129, 0:NC] = np.asarray(b2).astype(bf16)
    cvecn = np.zeros((1, 64), np.float32)
    t = np.arange(1, 51, dtype=np.float32)
    cvecn[0, :50] = -np.sqrt(2.0 * np.log(3.0) / t)
    return dict(W1s=W1T, wsm=wsm, cvecn=cvecn)


def _d025t(edge_index):
    ei = np.asarray(edge_index)
    deg = np.stack([np.clip(np.bincount(ei[r, 0], minlength=N).astype(np.float32), 1.0, None)
                    for r in range(NR)])
    d025 = deg ** -0.25
    d025p = np.zeros((NR, NPAD), np.float32)
    d025p[:, :N] = d025
    out = np.zeros((NR, RG_GROUPS, 128, 7), bf16)
    for r in range(NR):
        v = d025p[r].reshape(392, 128)
        out[r] = v.reshape(RG_GROUPS, 7, 128).transpose(0, 2, 1).astype(bf16)
    return out


def _build_dispatch(nc):
    """One-time construction of the jitted SPMD dispatch (cached across calls)."""
    import jax
    import jax.numpy as jnp
    from jax.experimental.shard_map import shard_map
    from jax.sharding import Mesh, PartitionSpec, NamedSharding
    from concourse import bass2jax

    bass2jax.install_neuronx_cc_hook()
    partition_name = nc.partition_id_tensor.name if nc.partition_id_tensor else None
    in_names, out_names, out_avals = [], [], []
    for alloc in nc.m.functions[0].allocations:
        if not isinstance(alloc, mybir.MemoryLocationSet):
            continue
        name = alloc.memorylocations[0].name
        if alloc.kind == "ExternalInput":
            if name != partition_name:
                in_names.append(name)
        elif alloc.kind == "ExternalOutput":
            shape = tuple(alloc.tensor_shape)
            dtype = mybir.dt.np(alloc.dtype)
            out_names.append(name)
            out_avals.append(jax.core.ShapedArray(shape, dtype))
    n_params = len(in_names)
    n_outs = len(out_avals)
    in_names_full = list(in_names) + list(out_names)
    if partition_name is not None:
        in_names_full.append(partition_name)
    donate = tuple(range(n_params, n_params + n_outs))

    def _body(*args):
        operands = list(args)
        if partition_name is not None:
            operands.append(bass2jax.partition_id_tensor())
        outs = bass2jax._bass_exec_p.bind(
            *operands, out_avals=tuple(out_avals), in_names=tuple(in_names_full),
            out_names=tuple(out_names), lowering_input_output_aliases=(),
            sim_require_finite=True, sim_require_nnan=True, nc=nc)
        return tuple(outs)

    devices = jax.devices()[:NCORES]
    mesh = Mesh(np.asarray(devices), ("core",))
    in_specs = (PartitionSpec("core"),) * (n_params + n_outs)
    # every core writes the identical full output (on-device allgather);
    # a replicated out_spec lets the host fetch a single device's copy
    out_specs = (PartitionSpec(),) * n_outs
    sharded = jax.jit(
        shard_map(_body, mesh=mesh, in_specs=in_specs, out_specs=out_specs,
                  check_rep=False),
        donate_argnums=donate, keep_unused=True)

    sh = NamedSharding(mesh, PartitionSpec("core"))
    zero_shapes = [(NCORES * a.shape[0], *a.shape[1:]) for a in out_avals]
    zero_dtypes = [a.dtype for a in out_avals]

    def _zeros():
        return tuple(jnp.zeros(s, d) for s, d in zip(zero_shapes, zero_dtypes))
    zeros_maker = jax.jit(_zeros, out_shardings=(sh,) * n_outs)
    return dict(in_names=in_names, out_names=out_names, out_avals=out_avals,
                sharded=sharded, zeros_maker=zeros_maker)


def kernel(x, edge_index, W1, b1, W2, b2):
    global LAST_SPMD_SECONDS
    import time as _time
    full = prepare(x, edge_index, W1, b1, W2, b2)
    shared = _shared_inputs(W1, b1, W2, b2)
    # W1s: global concat of per-core 1/8 row-slices == the full padded W1T
    full["W1s"] = shared["W1s"]
    # d025s: core c ships rescale-groups [c*GPL, (c+1)*GPL)
    d = _d025t(edge_index)                                # [NR, 56, 128, 7]
    full["d025s"] = np.ascontiguousarray(
        d.reshape(NR, NCORES, GPL, 128, 7).transpose(1, 0, 2, 3, 4)
    ).reshape(NCORES * NR, GPL, 128, 7)
    for k in ("wsm", "cvecn"):
        v = shared[k]
        full[k] = np.tile(v, (NCORES,) + (1,) * (v.ndim - 1))
    if "nc" not in _CACHED:
        _CACHED["nc"] = build_program()
    nc = _CACHED["nc"]
    try:
        if "disp" not in _CACHED:
            _CACHED["disp"] = _build_dispatch(nc)
        disp = _CACHED["disp"]
        in_names, out_names = disp["in_names"], disp["out_names"]
        import jax as _jax
        t0 = _time.time()
        concat_in = [full[name] for name in in_names]
        zo = disp["zeros_maker"]()
        outs = disp["sharded"](*concat_in, *zo)
        host = _jax.device_get(list(outs))
        LAST_SPMD_SECONDS = _time.time() - t0
        res = {name: np.asarray(host[i]) for i, name in enumerate(out_names)}
        both = res["out_all"][:N]
        lsm = both[:, 0:NC]
        logits = both[:, NC:2 * NC]
    except Exception:
        in_maps = []
        for c in range(NCORES):
            m = {}
            for k, v in full.items():
                p = v.shape[0] // NCORES
                m[k] = np.ascontiguousarray(v[c * p:(c + 1) * p])
            in_maps.append(m)
        t0 = _time.time()
        r = run_bass_kernel_spmd(nc, in_maps, core_ids=list(range(NCORES)))
        LAST_SPMD_SECONDS = _time.time() - t0
        both = r.results[0]["out_all"][:N]
        lsm = both[:, 0:NC]
        logits = both[:, NC:2 * NC]
    return lsm.astype(np.float32), logits.astype(np.float32)
